# revision 5
# baseline (speedup 1.0000x reference)
"""Causal multi-head attention (B=2, S=2048, D=1024, H=16) on 8 TRN2 NeuronCores.

Sharding: sequence-parallel. Cores 0-3 handle batch 0, cores 4-7 batch 1.
Within a batch group, core with local index l owns the mirrored pair of
256-row chunks (A = rows [256l, 256l+256), B = rows [256(7-l), 256(8-l))),
which equalizes causal attention work across cores (every core: 18 useful
kv-block jobs per head; we run a uniform 24-job structure so one SPMD
program serves all cores, with per-core host-computed masks selecting
valid/diagonal/invalid blocks).

Pipeline per core:
  1. x_local [512,1024] -> PE-transpose -> xT [d, s] layout in SBUF
  2. QKV projection: qT/kT in [c, s] layout (W as stationary), v in [s, c]
     (xT as stationary). Biases fused on DVE.
  3. kT+v AllGather across the 4-core batch group (one 4 MB collective)
  4. Per head-pair flash-style attention, scores computed transposed
     (scoresT[k, q]); softmax denominator via an appended ones-column in
     the PV matmul; no max-subtraction (scores are O(1) here).
  5. Output projection on the core's own rows; host reassembles.

All matmuls run in float32r (FP22 multipliers, fp32 accumulate): ~1e-3 rel
error vs fp32 reference, 4x faster than true fp32 on the PE.
"""

import numpy as np

B, S, D = 2, 2048, 1024
H = 16
HD = 64
NCORES = 8
CHUNK = 256          # rows per chunk; 2 chunks per core
SLOC = 2 * CHUNK     # rows per core
NPAIR = H // 2       # head pairs
NJOB = 24            # uniform job count per head: 16 B-phase + 8 A-phase
KT_ELEMS = D * SLOC  # fp32 elements of kT_local in cc buffer
V_ELEMS = SLOC * D

_CACHE = {}


def _build_nc():
    import concourse.bass as bass
    import concourse.bacc as bacc
    import concourse.mybir as mybir
    import concourse.tile as tile

    f32 = mybir.dt.float32
    f32r = mybir.dt.float32r
    bf16 = mybir.dt.bfloat16
    MULT = mybir.AluOpType.mult
    ADD = mybir.AluOpType.add
    EXP = mybir.ActivationFunctionType.Exp

    nc = bacc.Bacc(num_devices=NCORES)

    def dram_in(name, shape, dtype=f32r):
        return nc.dram_tensor(name, shape, dtype, kind="ExternalInput")

    x_in = dram_in("x_local", [SLOC, D])
    wqkv_in = dram_in("w_qkv", [D, 3 * D])
    bqk_in = dram_in("b_qk_t", [128, 16], f32)
    bv_in = dram_in("b_v_bc", [128, D])
    wout_in = dram_in("w_out", [D, D])
    masks_in = dram_in("masks", [128, NJOB, 2 * CHUNK], bf16)
    y_out = nc.dram_tensor("y", [SLOC, D], f32, kind="ExternalOutput")

    cc_in = nc.dram_tensor("cc_in", [KT_ELEMS + V_ELEMS], f32r)
    cc_out = nc.dram_tensor("cc_out", [4, KT_ELEMS + V_ELEMS], f32r)
    groups = [[0, 1, 2, 3], [4, 5, 6, 7]]

    def consts(name, arr):
        h = nc.inline_tensor(np.ascontiguousarray(arr, np.float32), name=name)
        return bass.DRamTensorHandle(h.name, h.shape, f32r)

    ident_c = consts("ident_c", np.eye(128, dtype=np.float32))
    zeros_c = consts("zeros_c", np.zeros((1, 512), np.float32))
    ones_c = consts("ones_c", np.ones((1, 64), np.float32))

    # DRAM views of gathered kv
    def kt_view(r):      # [D, SLOC] for rank r
        return cc_out[r, 0:KT_ELEMS].rearrange("(c s) -> c s", s=SLOC)

    def v_view(r):       # [SLOC, D] for rank r
        return cc_out[r, KT_ELEMS:].rearrange("(s c) -> s c", c=D)

    # chunk -> (owner rank, offset inside that rank's 512 rows)
    def chunk_owner(c):
        return (c, 0) if c < 4 else (7 - c, CHUNK)

    with tile.TileContext(nc) as tc:
        with tc.tile_pool(name="const", bufs=1) as cpool:
            it = cpool.tile([128, 128], f32r)
            nc.sync.dma_start(out=it[:], in_=ident_c[:])
            zt = cpool.tile([1, 512], f32r)
            nc.sync.dma_start(out=zt[:], in_=zeros_c[:])
            ones1 = cpool.tile([1, 64], f32r)
            nc.sync.dma_start(out=ones1[:], in_=ones_c[:])
            bqk = cpool.tile([128, 16], f32)
            nc.sync.dma_start(out=bqk[:], in_=bqk_in[:])
            bv = cpool.tile([128, D], f32r)
            nc.sync.dma_start(out=bv[:], in_=bv_in[:])
            masks = cpool.tile([128, NJOB, 2 * CHUNK], bf16)
            nc.sync.dma_start(out=masks[:], in_=masks_in[:])
            xT = cpool.tile([128, 8, SLOC], f32r)
            qT = cpool.tile([128, 8, SLOC], f32r)
            ctxT = cpool.tile([128, 8, SLOC], f32r)

            # ---- Phase 1: load x, transpose to xT ----
            with tc.tile_pool(name="ph1", bufs=3) as pool, \
                 tc.tile_pool(name="ph1p", bufs=4, space="PSUM") as psp:
                for sb in range(4):
                    xl = pool.tile([128, D], f32r, tag="xl")
                    nc.sync.dma_start(out=xl[:], in_=x_in[sb * 128:(sb + 1) * 128, :])
                    for db in range(8):
                        pst = psp.tile([128, 128], f32r, tag="tr")
                        nc.tensor.transpose(pst[:], xl[:, db * 128:(db + 1) * 128], it[:])
                        nc.vector.tensor_copy(out=xT[:, db, sb * 128:(sb + 1) * 128], in_=pst[:])

            # ---- Phase 2: QKV projection ----
            # W_qkv view [p, db, c]: row d = db*128+p
            wq_v = wqkv_in.rearrange("(db p) c -> p db c", p=128)
            with tc.tile_pool(name="ph2w", bufs=3) as wpool, \
                 tc.tile_pool(name="ph2wv", bufs=8) as wvpool, \
                 tc.tile_pool(name="ph2", bufs=3) as pool, \
                 tc.tile_pool(name="ph2p", bufs=2, space="PSUM") as psp:

                def qk_block(cb, out_sb):
                    # panel [128, 8, 128] = W_qkv[:, cb*128:(cb+1)*128] d-major
                    wp = wpool.tile([128, 8, 128], f32r, tag="wp", name=f"wp_{cb}")
                    nc.sync.dma_start(out=wp[:], in_=wq_v[:, :, cb * 128:(cb + 1) * 128])
                    ps = psp.tile([128, SLOC], f32, tag="qk", name=f"psqk_{cb}")
                    for db in range(8):
                        nc.tensor.matmul(ps[:], wp[:, db, :], xT[:, db, :],
                                         start=(db == 0), stop=(db == 7))
                    nc.vector.tensor_scalar_add(out_sb, ps[:], bqk[:, cb:cb + 1])

                # k blocks (c-blocks 8..15) -> cc_in kT region
                cc_kt = cc_in[0:KT_ELEMS].rearrange("(c s) -> c s", s=SLOC)
                for cb in range(8, 16):
                    kt = pool.tile([128, SLOC], f32r, tag="kt", name=f"kt_{cb}")
                    qk_block(cb, kt[:])
                    nc.sync.dma_start(out=cc_kt[(cb - 8) * 128:(cb - 7) * 128, :], in_=kt[:])

                # v blocks -> cc_in v region
                cc_v = cc_in[KT_ELEMS:].rearrange("(s c) -> s c", c=D)
                for nb in range(2):
                    wv = []
                    for db in range(8):
                        w = wvpool.tile([128, 512], f32r, tag="wv", name=f"wv_{nb}_{db}")
                        nc.sync.dma_start(
                            out=w[:],
                            in_=wqkv_in[db * 128:(db + 1) * 128,
                                        2 * D + nb * 512: 2 * D + (nb + 1) * 512])
                        wv.append(w)
                    for sb in range(4):
                        ps = psp.tile([128, 512], f32, tag="qk", name=f"psv_{nb}_{sb}")
                        for db in range(8):
                            nc.tensor.matmul(ps[:], xT[:, db, sb * 128:(sb + 1) * 128],
                                             wv[db][:], start=(db == 0), stop=(db == 7))
                        vt = pool.tile([128, 512], f32r, tag="vt", name=f"vt_{nb}_{sb}")
                        nc.vector.tensor_tensor(out=vt[:], in0=ps[:], in1=bv[:, nb * 512:(nb + 1) * 512], op=ADD)
                        nc.sync.dma_start(out=cc_v[sb * 128:(sb + 1) * 128, nb * 512:(nb + 1) * 512], in_=vt[:])

                # AllGather kT+v within batch group
                nc.gpsimd.collective_compute(
                    "AllGather", mybir.AluOpType.bypass, replica_groups=groups,
                    ins=[cc_in[:]], outs=[cc_out[:]],
                )

                # q blocks (c-blocks 0..7) -> qT resident (overlaps AllGather)
                for cb in range(8):
                    qk_block(cb, qT[:, cb, :])

            # ---- Phase 3: attention, one head-pair at a time ----
            with tc.tile_pool(name="kv", bufs=2) as kvpool, \
                 tc.tile_pool(name="at", bufs=4) as atpool, \
                 tc.tile_pool(name="misc", bufs=4) as mpool, \
                 tc.tile_pool(name="ps_s", bufs=4, space="PSUM") as spool, \
                 tc.tile_pool(name="ps_c", bufs=2, space="PSUM") as ctxpool, \
                 tc.tile_pool(name="ps_b", bufs=1, space="PSUM") as bpool:
                for p in range(NPAIR):
                    # kT_pair [128, 16, 128]: global kv-block order
                    ktp = kvpool.tile([128, 16, 128], f32r, tag="ktp")
                    for c in range(8):
                        r, off = chunk_owner(c)
                        nc.sync.dma_start(
                            out=ktp[:, 2 * c:2 * c + 2, :],
                            in_=kt_view(r)[p * 128:(p + 1) * 128, off:off + CHUNK]
                                .rearrange("p (b k) -> p b k", b=2),
                        )
                    # v_ext [128, 16, 130]: per slot [v_h0 | 1 | v_h1 | 1]
                    vxt = kvpool.tile([128, 16, 130], f32r, tag="vxt")
                    for c in range(8):
                        r, off = chunk_owner(c)
                        for h in range(2):
                            nc.sync.dma_start(
                                out=vxt[:, 2 * c:2 * c + 2, h * 65:h * 65 + 64],
                                in_=v_view(r)[off:off + CHUNK, (2 * p + h) * 64:(2 * p + h + 1) * 64]
                                    .rearrange("(o q) k -> q o k", q=128),
                            )
                    ones_col = mpool.tile([128, 16, 2], f32, tag="onescol")
                    nc.vector.memset(ones_col[:], 1.0)
                    nc.vector.tensor_copy(out=vxt[:, :, 64:65], in_=ones_col[:, :, 0:1])
                    nc.vector.tensor_copy(out=vxt[:, :, 129:130], in_=ones_col[:, :, 1:2])

                    psc = [ctxpool.tile([65, 512], f32, tag="ctx", name=f"ctx_p{p}_{j}") for j in range(2)]
                    for h in range(2):
                        nc.tensor.matmul(psc[h][:, :], zt[0:1, 0:65], zt[0:1, 0:512],
                                         start=True, stop=False, skip_group_check=True)

                    for i in range(NJOB):
                        kv = (15 - i) if i < 16 else (23 - i)
                        choff = CHUNK if i < 16 else 0  # B jobs hit cols 256:512
                        pss = [None, None]
                        ats = [None, None]
                        for h in range(2):
                            pss[h] = spool.tile([128, CHUNK], f32, tag="s", name=f"s_{p}_{i}_{h}")
                            nc.tensor.matmul(
                                pss[h][:],
                                ktp[h * 64:(h + 1) * 64, kv, :],
                                qT[h * 64:(h + 1) * 64, p, choff:choff + CHUNK],
                                start=True, stop=True, tile_position=(h * 64, 0),
                            )
                        for h in range(2):
                            ats[h] = atpool.tile([128, CHUNK], f32r, tag="at", name=f"at_{p}_{i}_{h}")
                            nc.scalar.activation(ats[h][:], pss[h][:], EXP, scale=0.125)
                            nc.vector.tensor_tensor(
                                out=ats[h][:], in0=ats[h][:],
                                in1=masks[:, i, choff:choff + CHUNK], op=MULT)
                        for h in range(2):
                            nc.tensor.matmul(
                                psc[h][:, choff:choff + CHUNK],
                                vxt[:, kv, h * 65:h * 65 + 65],
                                ats[h][:],
                                start=False, stop=(i == NJOB - 1), skip_group_check=True,
                            )

                    for h in range(2):
                        recip = mpool.tile([1, 512], f32r, tag="recip")
                        with nc.allow_low_precision(reason="f32r==f32 bits"):
                            nc.vector.reciprocal(recip[:], psc[h][64:65, :])
                        psb = bpool.tile([64, 512], f32, tag="b")
                        nc.tensor.matmul(psb[:], ones1[0:1, :], recip[0:1, :],
                                         start=True, stop=True)
                        csb = mpool.tile([64, 512], f32, tag="csb")
                        nc.vector.tensor_copy(out=csb[:], in_=psc[h][0:64, :])
                        nc.vector.tensor_tensor(
                            out=ctxT[h * 64:(h + 1) * 64, p, :],
                            in0=csb[:], in1=psb[:], op=MULT)

            # ---- Phase 4: output projection ----
            with tc.tile_pool(name="ph4w", bufs=8) as wpool, \
                 tc.tile_pool(name="ph4", bufs=3) as pool, \
                 tc.tile_pool(name="ph4p", bufs=2, space="PSUM") as psp:
                wo = []
                for cb in range(8):
                    w = wpool.tile([128, D], f32r, tag="wo", name=f"wo_{cb}")
                    nc.sync.dma_start(out=w[:], in_=wout_in[cb * 128:(cb + 1) * 128, :])
                    wo.append(w)
                for sb in range(4):
                    for nb in range(2):
                        ps = psp.tile([128, 512], f32, tag="y")
                        for cb in range(8):
                            nc.tensor.matmul(ps[:], ctxT[:, cb, sb * 128:(sb + 1) * 128],
                                             wo[cb][:, nb * 512:(nb + 1) * 512],
                                             start=(cb == 0), stop=(cb == 7))
                        yt = pool.tile([128, 512], f32, tag="yt")
                        nc.vector.tensor_copy(out=yt[:], in_=ps[:])
                        nc.sync.dma_start(
                            out=y_out[sb * 128:(sb + 1) * 128, nb * 512:(nb + 1) * 512],
                            in_=yt[:])

    nc.finalize()
    return nc


def _host_inputs(x, W_qkv, b_qkv, W_out):
    import ml_dtypes

    x = np.asarray(x, np.float32)
    W_qkv = np.ascontiguousarray(np.asarray(W_qkv, np.float32))
    b_qkv = np.asarray(b_qkv, np.float32)
    W_out = np.ascontiguousarray(np.asarray(W_out, np.float32))

    bqk_t = np.ascontiguousarray(b_qkv[:2 * D].reshape(16, 128).T)  # [128, 16]
    bv_bc = np.ascontiguousarray(np.broadcast_to(b_qkv[2 * D:], (128, D)))

    in_maps = []
    for c in range(NCORES):
        b, l = divmod(c, 4)
        cA, cB = l, 7 - l
        x_local = np.ascontiguousarray(
            np.concatenate([x[b, cA * CHUNK:(cA + 1) * CHUNK],
                            x[b, cB * CHUNK:(cB + 1) * CHUNK]], axis=0))
        # masks [128, 24, 512]: job i -> [p, i, f]; A jobs cols 0:256, B cols 256:512
        m = np.zeros((128, NJOB, 2 * CHUNK), np.float32)
        pp = np.arange(128)[:, None]
        ff = np.arange(CHUNK)[None, :]
        for i in range(NJOB):
            if i < 16:
                kvb, r0, sl = 15 - i, cB * CHUNK, slice(CHUNK, 2 * CHUNK)
            else:
                kvb, r0, sl = 23 - i, cA * CHUNK, slice(0, CHUNK)
            m[:, i, sl] = (128 * kvb + pp <= r0 + ff).astype(np.float32)
        in_maps.append({
            "x_local": x_local,
            "w_qkv": W_qkv,
            "b_qk_t": bqk_t,
            "b_v_bc": bv_bc,
            "w_out": W_out,
            "masks": m.astype(ml_dtypes.bfloat16),
        })
    return in_maps


def _run(in_maps, trace=False):
    from concourse.bass_utils import run_bass_kernel_spmd

    if "nc" not in _CACHE:
        _CACHE["nc"] = _build_nc()
    return run_bass_kernel_spmd(_CACHE["nc"], in_maps, core_ids=list(range(NCORES)),
                                trace=trace)


def kernel(x, W_qkv, b_qkv, W_out):
    in_maps = _host_inputs(x, W_qkv, b_qkv, W_out)
    res = _run(in_maps)
    out = np.empty((B, S, D), np.float32)
    for c in range(NCORES):
        b, l = divmod(c, 4)
        y = res.results[c]["y"]
        out[b, l * CHUNK:(l + 1) * CHUNK] = y[0:CHUNK]
        out[b, (7 - l) * CHUNK:(8 - l) * CHUNK] = y[CHUNK:2 * CHUNK]
    return out


# revision 6
# speedup vs baseline: 1.3581x; 1.3581x over previous
"""Causal multi-head attention (B=2, S=2048, D=1024, H=16) on 8 TRN2 NeuronCores.

Sharding: sequence-parallel. Cores 0-3 handle batch 0, cores 4-7 batch 1.
Within a batch group, core with local index l owns the mirrored pair of
256-row chunks (A = rows [256l, 256l+256), B = rows [256(7-l), 256(8-l))),
which equalizes causal attention work across cores (every core: 18 useful
kv-block jobs per head; we run a uniform 24-job structure so one SPMD
program serves all cores, with per-core host-computed masks selecting
valid/diagonal/invalid blocks).

Pipeline per core:
  1. x_local [512,1024] -> PE-transpose -> xT [d, s] layout in SBUF
  2. QKV projection: qT/kT in [c, s] layout (W as stationary), v in [s, c]
     (xT as stationary). Biases fused on DVE.
  3. kT+v AllGather across the 4-core batch group (one 4 MB collective)
  4. Per head-pair flash-style attention, scores computed transposed
     (scoresT[k, q]); softmax denominator via an appended ones-column in
     the PV matmul; no max-subtraction (scores are O(1) here).
  5. Output projection on the core's own rows; host reassembles.

All matmuls run in float32r (FP22 multipliers, fp32 accumulate): ~1e-3 rel
error vs fp32 reference, 4x faster than true fp32 on the PE.
"""

import numpy as np

B, S, D = 2, 2048, 1024
H = 16
HD = 64
NCORES = 8
CHUNK = 256          # rows per chunk; 2 chunks per core
SLOC = 2 * CHUNK     # rows per core
NPAIR = H // 2       # head pairs
NJOB = 24            # uniform job count per head: 16 B-phase + 8 A-phase
KT_ELEMS = D * SLOC  # fp32 elements of kT_local in cc buffer
V_ELEMS = SLOC * D

_CACHE = {}


def _build_nc():
    import concourse.bass as bass
    import concourse.bacc as bacc
    import concourse.mybir as mybir
    import concourse.tile as tile

    f32 = mybir.dt.float32
    f32r = mybir.dt.float32r
    bf16 = mybir.dt.bfloat16
    MULT = mybir.AluOpType.mult
    ADD = mybir.AluOpType.add
    EXP = mybir.ActivationFunctionType.Exp

    nc = bacc.Bacc(num_devices=NCORES)

    def dram_in(name, shape, dtype=f32r):
        return nc.dram_tensor(name, shape, dtype, kind="ExternalInput")

    x_in = dram_in("x_local", [SLOC, D], bf16)
    wqkv_in = dram_in("w_qkv", [D, 3 * D], bf16)
    bqk_in = dram_in("b_qk_t", [128, 16], f32)
    bv_in = dram_in("b_v_bc", [128, D], f32)
    wout_in = dram_in("w_out", [D, D], bf16)
    masks_in = dram_in("masks", [128, NJOB, 2 * CHUNK], bf16)
    y_out = nc.dram_tensor("y", [SLOC, D], f32, kind="ExternalOutput")

    cc_in = nc.dram_tensor("cc_in", [KT_ELEMS + V_ELEMS], bf16)
    cc_out = nc.dram_tensor("cc_out", [4, KT_ELEMS + V_ELEMS], bf16)
    groups = [[0, 1, 2, 3], [4, 5, 6, 7]]

    import ml_dtypes

    def consts(name, arr, dt_np, dt_bass):
        h = nc.inline_tensor(np.ascontiguousarray(arr, dt_np), name=name)
        if dt_bass is None:
            return h
        return bass.DRamTensorHandle(h.name, h.shape, dt_bass)

    ident_c = consts("ident_c", np.eye(128), ml_dtypes.bfloat16, None)
    zeros_c = consts("zeros_c", np.zeros((1, 512)), ml_dtypes.bfloat16, None)
    ones_c = consts("ones_c", np.ones((1, 64)), np.float32, f32r)

    # DRAM views of gathered kv
    def kt_view(r):      # [D, SLOC] for rank r
        return cc_out[r, 0:KT_ELEMS].rearrange("(c s) -> c s", s=SLOC)

    def v_view(r):       # [SLOC, D] for rank r
        return cc_out[r, KT_ELEMS:].rearrange("(s c) -> s c", c=D)

    # chunk -> (owner rank, offset inside that rank's 512 rows)
    def chunk_owner(c):
        return (c, 0) if c < 4 else (7 - c, CHUNK)

    with tile.TileContext(nc) as tc:
        with tc.tile_pool(name="const", bufs=1) as cpool:
            it = cpool.tile([128, 128], bf16)
            nc.sync.dma_start(out=it[:], in_=ident_c[:])
            zt = cpool.tile([1, 512], bf16)
            nc.sync.dma_start(out=zt[:], in_=zeros_c[:])
            ones1 = cpool.tile([1, 64], f32r)
            nc.sync.dma_start(out=ones1[:], in_=ones_c[:])
            bqk = cpool.tile([128, 16], f32)
            nc.sync.dma_start(out=bqk[:], in_=bqk_in[:])
            bv = cpool.tile([128, D], f32)
            nc.sync.dma_start(out=bv[:], in_=bv_in[:])
            masks = cpool.tile([128, NJOB, 2 * CHUNK], bf16)
            nc.sync.dma_start(out=masks[:], in_=masks_in[:])
            xT = cpool.tile([128, 8, SLOC], bf16)
            qT = cpool.tile([128, 8, SLOC], bf16)
            ctxT = cpool.tile([128, 8, SLOC], bf16)

            # ---- Phase 1: load x, transpose to xT ----
            with tc.tile_pool(name="ph1", bufs=3) as pool, \
                 tc.tile_pool(name="ph1p", bufs=4, space="PSUM") as psp:
                for sb in range(4):
                    xl = pool.tile([128, D], bf16, tag="xl")
                    nc.sync.dma_start(out=xl[:], in_=x_in[sb * 128:(sb + 1) * 128, :])
                    for db in range(8):
                        pst = psp.tile([128, 128], bf16, tag="tr")
                        nc.tensor.transpose(pst[:], xl[:, db * 128:(db + 1) * 128], it[:])
                        nc.vector.tensor_copy(out=xT[:, db, sb * 128:(sb + 1) * 128], in_=pst[:])

            # ---- Phase 2: QKV projection ----
            # W_qkv view [p, db, c]: row d = db*128+p
            wq_v = wqkv_in.rearrange("(db p) c -> p db c", p=128)
            with tc.tile_pool(name="ph2w", bufs=3) as wpool, \
                 tc.tile_pool(name="ph2wv", bufs=8) as wvpool, \
                 tc.tile_pool(name="ph2", bufs=3) as pool, \
                 tc.tile_pool(name="ph2p", bufs=2, space="PSUM") as psp:

                def qk_block(cb, out_sb):
                    # panel [128, 8, 128] = W_qkv[:, cb*128:(cb+1)*128] d-major
                    wp = wpool.tile([128, 8, 128], bf16, tag="wp", name=f"wp_{cb}")
                    nc.sync.dma_start(out=wp[:], in_=wq_v[:, :, cb * 128:(cb + 1) * 128])
                    ps = psp.tile([128, SLOC], f32, tag="qk", name=f"psqk_{cb}")
                    for db in range(8):
                        nc.tensor.matmul(ps[:], wp[:, db, :], xT[:, db, :],
                                         start=(db == 0), stop=(db == 7))
                    nc.vector.tensor_scalar_add(out_sb, ps[:], bqk[:, cb:cb + 1])

                # k blocks (c-blocks 8..15) -> cc_in kT region
                cc_kt = cc_in[0:KT_ELEMS].rearrange("(c s) -> c s", s=SLOC)
                for cb in range(8, 16):
                    kt = pool.tile([128, SLOC], bf16, tag="kt", name=f"kt_{cb}")
                    qk_block(cb, kt[:])
                    nc.sync.dma_start(out=cc_kt[(cb - 8) * 128:(cb - 7) * 128, :], in_=kt[:])

                # v blocks -> cc_in v region
                cc_v = cc_in[KT_ELEMS:].rearrange("(s c) -> s c", c=D)
                for nb in range(2):
                    wv = []
                    for db in range(8):
                        w = wvpool.tile([128, 512], bf16, tag="wv", name=f"wv_{nb}_{db}")
                        nc.sync.dma_start(
                            out=w[:],
                            in_=wqkv_in[db * 128:(db + 1) * 128,
                                        2 * D + nb * 512: 2 * D + (nb + 1) * 512])
                        wv.append(w)
                    for sb in range(4):
                        ps = psp.tile([128, 512], f32, tag="qk", name=f"psv_{nb}_{sb}")
                        for db in range(8):
                            nc.tensor.matmul(ps[:], xT[:, db, sb * 128:(sb + 1) * 128],
                                             wv[db][:], start=(db == 0), stop=(db == 7))
                        vt = pool.tile([128, 512], bf16, tag="vt", name=f"vt_{nb}_{sb}")
                        nc.vector.tensor_tensor(out=vt[:], in0=ps[:], in1=bv[:, nb * 512:(nb + 1) * 512], op=ADD)
                        nc.sync.dma_start(out=cc_v[sb * 128:(sb + 1) * 128, nb * 512:(nb + 1) * 512], in_=vt[:])

                # AllGather kT+v within batch group
                nc.gpsimd.collective_compute(
                    "AllGather", mybir.AluOpType.bypass, replica_groups=groups,
                    ins=[cc_in[:]], outs=[cc_out[:]],
                )

                # q blocks (c-blocks 0..7) -> qT resident (overlaps AllGather)
                for cb in range(8):
                    qk_block(cb, qT[:, cb, :])

            # ---- Phase 3: attention, one head-pair at a time ----
            with tc.tile_pool(name="kv", bufs=2) as kvpool, \
                 tc.tile_pool(name="at", bufs=4) as atpool, \
                 tc.tile_pool(name="misc", bufs=4) as mpool, \
                 tc.tile_pool(name="ps_s", bufs=4, space="PSUM") as spool, \
                 tc.tile_pool(name="ps_c", bufs=2, space="PSUM") as ctxpool, \
                 tc.tile_pool(name="ps_b", bufs=1, space="PSUM") as bpool:
                for p in range(NPAIR):
                    # kT_pair [128, 16, 128]: global kv-block order
                    ktp = kvpool.tile([128, 16, 128], bf16, tag="ktp")
                    for c in range(8):
                        r, off = chunk_owner(c)
                        nc.sync.dma_start(
                            out=ktp[:, 2 * c:2 * c + 2, :],
                            in_=kt_view(r)[p * 128:(p + 1) * 128, off:off + CHUNK]
                                .rearrange("p (b k) -> p b k", b=2),
                        )
                    # v_ext [128, 16, 130]: per slot [v_h0 | 1 | v_h1 | 1]
                    vxt = kvpool.tile([128, 16, 130], bf16, tag="vxt")
                    for c in range(8):
                        r, off = chunk_owner(c)
                        for h in range(2):
                            nc.sync.dma_start(
                                out=vxt[:, 2 * c:2 * c + 2, h * 65:h * 65 + 64],
                                in_=v_view(r)[off:off + CHUNK, (2 * p + h) * 64:(2 * p + h + 1) * 64]
                                    .rearrange("(o q) k -> q o k", q=128),
                            )
                    ones_col = mpool.tile([128, 16, 2], bf16, tag="onescol")
                    nc.vector.memset(ones_col[:], 1.0)
                    nc.vector.tensor_copy(out=vxt[:, :, 64:65], in_=ones_col[:, :, 0:1])
                    nc.vector.tensor_copy(out=vxt[:, :, 129:130], in_=ones_col[:, :, 1:2])

                    psc = [ctxpool.tile([65, 512], f32, tag="ctx", name=f"ctx_p{p}_{j}") for j in range(2)]
                    for h in range(2):
                        nc.tensor.matmul(psc[h][:, :], zt[0:1, 0:65], zt[0:1, 0:512],
                                         start=True, stop=False, skip_group_check=True)

                    for i in range(NJOB):
                        kv = (15 - i) if i < 16 else (23 - i)
                        choff = CHUNK if i < 16 else 0  # B jobs hit cols 256:512
                        pss = [None, None]
                        ats = [None, None]
                        for h in range(2):
                            pss[h] = spool.tile([128, CHUNK], f32, tag="s", name=f"s_{p}_{i}_{h}")
                            nc.tensor.matmul(
                                pss[h][:],
                                ktp[h * 64:(h + 1) * 64, kv, :],
                                qT[h * 64:(h + 1) * 64, p, choff:choff + CHUNK],
                                start=True, stop=True, tile_position=(h * 64, 0),
                            )
                        for h in range(2):
                            ats[h] = atpool.tile([128, CHUNK], bf16, tag="at", name=f"at_{p}_{i}_{h}")
                            nc.scalar.activation(ats[h][:], pss[h][:], EXP, scale=0.125)
                            nc.vector.tensor_tensor(
                                out=ats[h][:], in0=ats[h][:],
                                in1=masks[:, i, choff:choff + CHUNK], op=MULT)
                        for h in range(2):
                            nc.tensor.matmul(
                                psc[h][:, choff:choff + CHUNK],
                                vxt[:, kv, h * 65:h * 65 + 65],
                                ats[h][:],
                                start=False, stop=(i == NJOB - 1), skip_group_check=True,
                            )

                    for h in range(2):
                        recip = mpool.tile([1, 512], f32r, tag="recip")
                        with nc.allow_low_precision(reason="f32r==f32 bits"):
                            nc.vector.reciprocal(recip[:], psc[h][64:65, :])
                        psb = bpool.tile([64, 512], f32, tag="b")
                        nc.tensor.matmul(psb[:], ones1[0:1, :], recip[0:1, :],
                                         start=True, stop=True)
                        csb = mpool.tile([64, 512], f32, tag="csb")
                        nc.vector.tensor_copy(out=csb[:], in_=psc[h][0:64, :])
                        nc.vector.tensor_tensor(
                            out=ctxT[h * 64:(h + 1) * 64, p, :],
                            in0=csb[:], in1=psb[:], op=MULT)

            # ---- Phase 4: output projection ----
            with tc.tile_pool(name="ph4w", bufs=8) as wpool, \
                 tc.tile_pool(name="ph4", bufs=3) as pool, \
                 tc.tile_pool(name="ph4p", bufs=2, space="PSUM") as psp:
                wo = []
                for cb in range(8):
                    w = wpool.tile([128, D], bf16, tag="wo", name=f"wo_{cb}")
                    nc.sync.dma_start(out=w[:], in_=wout_in[cb * 128:(cb + 1) * 128, :])
                    wo.append(w)
                for sb in range(4):
                    for nb in range(2):
                        ps = psp.tile([128, 512], f32, tag="y")
                        for cb in range(8):
                            nc.tensor.matmul(ps[:], ctxT[:, cb, sb * 128:(sb + 1) * 128],
                                             wo[cb][:, nb * 512:(nb + 1) * 512],
                                             start=(cb == 0), stop=(cb == 7))
                        yt = pool.tile([128, 512], f32, tag="yt")
                        nc.vector.tensor_copy(out=yt[:], in_=ps[:])
                        nc.sync.dma_start(
                            out=y_out[sb * 128:(sb + 1) * 128, nb * 512:(nb + 1) * 512],
                            in_=yt[:])

    nc.finalize()
    return nc


def _host_inputs(x, W_qkv, b_qkv, W_out):
    import ml_dtypes

    x = np.asarray(x, ml_dtypes.bfloat16)
    W_qkv = np.ascontiguousarray(np.asarray(W_qkv, ml_dtypes.bfloat16))
    b_qkv = np.asarray(b_qkv, np.float32)
    W_out = np.ascontiguousarray(np.asarray(W_out, ml_dtypes.bfloat16))

    bqk_t = np.ascontiguousarray(b_qkv[:2 * D].reshape(16, 128).T)  # [128, 16]
    bv_bc = np.ascontiguousarray(np.broadcast_to(b_qkv[2 * D:], (128, D)))

    in_maps = []
    for c in range(NCORES):
        b, l = divmod(c, 4)
        cA, cB = l, 7 - l
        x_local = np.ascontiguousarray(
            np.concatenate([x[b, cA * CHUNK:(cA + 1) * CHUNK],
                            x[b, cB * CHUNK:(cB + 1) * CHUNK]], axis=0))  # bf16
        # masks [128, 24, 512]: job i -> [p, i, f]; A jobs cols 0:256, B cols 256:512
        m = np.zeros((128, NJOB, 2 * CHUNK), np.float32)
        pp = np.arange(128)[:, None]
        ff = np.arange(CHUNK)[None, :]
        for i in range(NJOB):
            if i < 16:
                kvb, r0, sl = 15 - i, cB * CHUNK, slice(CHUNK, 2 * CHUNK)
            else:
                kvb, r0, sl = 23 - i, cA * CHUNK, slice(0, CHUNK)
            m[:, i, sl] = (128 * kvb + pp <= r0 + ff).astype(np.float32)
        in_maps.append({
            "x_local": x_local,
            "w_qkv": W_qkv,
            "b_qk_t": bqk_t,
            "b_v_bc": bv_bc,
            "w_out": W_out,
            "masks": m.astype(ml_dtypes.bfloat16),
        })
    return in_maps


def _run(in_maps, trace=False):
    from concourse.bass_utils import run_bass_kernel_spmd

    if "nc" not in _CACHE:
        _CACHE["nc"] = _build_nc()
    return run_bass_kernel_spmd(_CACHE["nc"], in_maps, core_ids=list(range(NCORES)),
                                trace=trace)


def kernel(x, W_qkv, b_qkv, W_out):
    in_maps = _host_inputs(x, W_qkv, b_qkv, W_out)
    res = _run(in_maps)
    out = np.empty((B, S, D), np.float32)
    for c in range(NCORES):
        b, l = divmod(c, 4)
        y = res.results[c]["y"]
        out[b, l * CHUNK:(l + 1) * CHUNK] = y[0:CHUNK]
        out[b, (7 - l) * CHUNK:(8 - l) * CHUNK] = y[CHUNK:2 * CHUNK]
    return out
